# revision 2
# baseline (speedup 1.0000x reference)
"""ConvMod3d (StyleGAN-style modulated 3x3x3 conv, N=4 groups) on 8 trn2 cores.

Sharding: 8 shards = 4 samples x 2 h-row halves. Each core computes ALL 46
output planes of its sample but only 23 of the 46 output h-rows (load-
balanced; the d-split alternative leaves one core a half-utilized group).
Style modulation/demodulation of the tiny weight tensor happens on host;
the conv (99.8% of FLOPs) on device.

PE packing: the 27 taps are covered per output plane by 13 contract-128
matmuls + 1 contract-64 (vs baseline's 12+3), using three SBUF window
families over the per-plane slabs (26 h-rows x 48 = 1248 cols):
- w[d]  = plane d | plane d+1           -> 9 (kd0,kd1) tap dominoes
- w2[p] = plane p | plane p (+48 cols)  -> 3 (kd2: kh0,kh1) dominoes,
          and its lower half @+98 feeds the lone c64 tap (kd2,kh2,kw2)
- w3[p] = plane p (+96) | plane p (+97) -> 1 (kd2,kh2: kw0,kw1) domino.
  w3 is built ON-CHIP from w2 by two same-partition vector/gpsimd copies
  (lower from w2 lower @+96, upper from w2 upper @+49) - loading it from
  HBM was tried in a previous session and stalled the PE on window waits.
Two output planes run concurrently on PE col strips (ci=0 in one PSUM
bank partitions 0-63, ci=1 in another, partitions 64-127), sharing each
weight block. Matmuls in bf16 (fp32 PSUM accumulation).

Free dim per plane = 23 out rows x 48 cols = 1104, chunked 3x368 (uniform
chunks beat 512+512+80: slots below ~256 rows go LDWEIGHTS-bound; 368-row
slots are stream-bound at ~155ns). Serpentine j-order across chunks keeps
the single c64 slot adjacent to its neighbor chunk's c64 slot. Startup
loads group 0's windows in split pieces so the first matmul issues ~2.5us
in; output DMA is per-group so the drain is one act pair + one 565KB DMA.
"""

import time

import numpy as np
import ml_dtypes

import concourse.bacc as bacc
import concourse.bass as bass
import concourse.tile as tile
from concourse import mybir
from concourse.bass_utils import run_bass_kernel_spmd

EPS = 1e-8
N, CIN, COUT = 4, 64, 64
DHW, K = 48, 3
DOUT = DHW - K + 1          # 46 output planes, all on each core
ROWS_PER_CORE = 23          # output h-rows per core
SLAB_ROWS = ROWS_PER_CORE + K  # 26 input h-rows per plane slab
SLAB = SLAB_ROWS * DHW      # 1248 cols per plane slab
NPLANES = DHW               # 48 input planes per core
XS_COLS = NPLANES * SLAB
WC = 1216                   # w / w2 window cols (max read offset 1202)
W2_UP = 1200                # w2 upper half cols (source slab ends at 1248)
W3C = 1120                  # w3 window cols (max read offset 1104)
FREE = ROWS_PER_CORE * DHW  # 1104 computed output cols (w' 46,47 dropped)
CH = 368                    # chunk cols (3 uniform chunks per group)
NCH = FREE // CH
NG = DOUT // 2              # 23 groups of 2 output planes
NJ = 14                     # weight blocks: 13 c128 dominoes + 1 c64 mono
NCORES = 8
LOOKAHEAD = 4               # groups of windows prefetched ahead

F32 = mybir.dt.float32
MM_DT = mybir.dt.bfloat16
NP_MM = np.dtype(ml_dtypes.bfloat16)

_CACHE = {}
LAST_RESULTS = None  # BassKernelResults of the most recent device run


def _build_bass():
    nc = bacc.Bacc()
    xs = nc.declare_dram_parameter("xs", [CIN, XS_COLS], MM_DT, isOutput=False)
    wt = nc.declare_dram_parameter("wt", [128, NJ * COUT], MM_DT, isOutput=False)
    bt = nc.declare_dram_parameter("bt", [128, 1], F32, isOutput=False)
    y = nc.declare_dram_parameter("y", [NG, 128, FREE], F32, isOutput=True)

    with tile.TileContext(nc) as tc:
        with (
            tc.tile_pool(name="const", bufs=1) as cpool,
            tc.tile_pool(name="wpool", bufs=12) as wpool,
            tc.tile_pool(name="w2pool", bufs=12) as w2pool,
            tc.tile_pool(name="w3pool", bufs=12) as w3pool,
            tc.tile_pool(name="opool", bufs=3) as opool,
            tc.tile_pool(name="ppool", bufs=8, space="PSUM") as ppool,
        ):
            wtile = cpool.tile([128, NJ * COUT], MM_DT)
            nc.sync.dma_start(out=wtile[:, :], in_=wt[:, :])
            btile = cpool.tile([128, 1], F32)
            nc.sync.dma_start(out=btile[:, :], in_=bt[:, :])

            W, W2, W3 = {}, {}, {}

            def pieces(tot, split):
                cuts = [0, 512, tot] if split else [0, tot]
                return list(zip(cuts, cuts[1:]))

            def load_w(d, split=False):
                if d in W or d >= DOUT:
                    return
                t = wpool.tile([128, WC], MM_DT, tag="w", name="w")
                base = d * SLAB
                for a, b in pieces(WC, split):
                    nc.sync.dma_start(out=t[0:64, a:b],
                                      in_=xs[:, base + a:base + b])
                    nc.sync.dma_start(out=t[64:128, a:b],
                                      in_=xs[:, base + SLAB + a:base + SLAB + b])
                W[d] = t

            def load_w2(p, split=False):
                if p in W2 or p >= NPLANES:
                    return
                t = w2pool.tile([128, WC], MM_DT, tag="w2", name="w2")
                base = p * SLAB
                for a, b in pieces(WC, split):
                    nc.sync.dma_start(out=t[0:64, a:b],
                                      in_=xs[:, base + a:base + b])
                for a, b in pieces(W2_UP, split):
                    nc.sync.dma_start(out=t[64:128, a:b],
                                      in_=xs[:, base + 48 + a:base + 48 + b])
                W2[p] = t

            def make_w3(p):
                # built on-chip: both halves are same-partition shifted
                # copies out of w2[p] (vector+gpsimd, both otherwise idle)
                if p in W3 or p >= NPLANES:
                    return
                s = W2[p]
                t = w3pool.tile([128, W3C], MM_DT, tag="w3", name="w3")
                nc.vector.tensor_copy(t[0:64, 0:W3C], s[0:64, 96:96 + W3C])
                nc.gpsimd.tensor_copy(t[64:128, 0:W3C], s[64:128, 49:49 + W3C])
                W3[p] = t

            def ensure_group(g, split=False):
                if g >= NG:
                    return
                for d in (2 * g, 2 * g + 1):
                    load_w(d, split=split)
                    load_w2(d + 2, split=split)
                    make_w3(d + 2)

            ensure_group(0, split=True)
            for g0 in range(1, LOOKAHEAD):
                ensure_group(g0)

            for g in range(NG):
                ensure_group(g + LOOKAHEAD)
                ot = opool.tile([128, FREE], F32, tag="ot")
                for cidx in range(NCH):
                    c0 = cidx * CH
                    pss = [ppool.tile([128, 512], F32, tag="ps", name="ps")
                           for _ in range(2)]
                    jorder = list(range(NJ))
                    if (g * NCH + cidx) % 2 == 1:
                        jorder = jorder[::-1]
                    mms = [(j, ci) for j in jorder for ci in range(2)]
                    first_ci, last_ci = {}, {}
                    for idx, (j, ci) in enumerate(mms):
                        first_ci.setdefault(ci, idx)
                        last_ci[ci] = idx
                    for idx, (j, ci) in enumerate(mms):
                        d = 2 * g + ci
                        if j < 9:
                            kh, kw = divmod(j, 3)
                            win, off, rows = W[d], kh * DHW + kw + c0, 128
                        elif j < 12:
                            win, off, rows = W2[d + 2], (j - 9) + c0, 128
                        elif j == 12:
                            win, off, rows = W3[d + 2], c0, 128
                        else:
                            win, off, rows = W2[d + 2], 2 * DHW + 2 + c0, 64
                        nc.tensor.matmul(
                            pss[ci][ci * 64:(ci + 1) * 64, 0:CH],
                            wtile[0:rows, j * 64:(j + 1) * 64],
                            win[0:rows, off:off + CH],
                            start=(idx == first_ci[ci]),
                            stop=(idx == last_ci[ci]),
                        )
                    for ci in range(2):
                        nc.scalar.activation(
                            ot[ci * 64:(ci + 1) * 64, c0:c0 + CH],
                            pss[ci][ci * 64:(ci + 1) * 64, 0:CH],
                            mybir.ActivationFunctionType.Identity,
                            bias=btile[ci * 64:(ci + 1) * 64, :],
                        )
                nc.scalar.dma_start(out=y[g, :, :], in_=ot[:, :])
    nc.compile()
    return nc


def _prep_in_maps(x, s, style_weight, style_bias, weight, bias):
    style = s @ style_weight.T + style_bias                      # [N, Cin]
    wm = weight[None] * style[:, None, :, None, None, None]      # [N,Co,Ci,k,k,k]
    wm = wm * (1.0 / np.sqrt((wm * wm).sum(axis=(2, 3, 4, 5), keepdims=True) + EPS))
    wk = wm.transpose(0, 2, 3, 4, 5, 1)                          # [N,Ci,kd,kh,kw,Co]
    wfull = np.zeros((N, 128, NJ * COUT), np.float32)
    for j in range(9):
        kh, kw = divmod(j, 3)
        wfull[:, 0:64, j * 64:(j + 1) * 64] = wk[:, :, 0, kh, kw, :]
        wfull[:, 64:128, j * 64:(j + 1) * 64] = wk[:, :, 1, kh, kw, :]
    for kw in range(3):
        j = 9 + kw
        wfull[:, 0:64, j * 64:(j + 1) * 64] = wk[:, :, 2, 0, kw, :]
        wfull[:, 64:128, j * 64:(j + 1) * 64] = wk[:, :, 2, 1, kw, :]
    wfull[:, 0:64, 12 * 64:13 * 64] = wk[:, :, 2, 2, 0, :]
    wfull[:, 64:128, 12 * 64:13 * 64] = wk[:, :, 2, 2, 1, :]
    wfull[:, 0:64, 13 * 64:14 * 64] = wk[:, :, 2, 2, 2, :]
    wfull = np.ascontiguousarray(wfull.astype(NP_MM))
    bt = np.ascontiguousarray(
        np.tile(bias[:, None], (2, 1)), dtype=np.float32)        # [128,1]

    in_maps = []
    for core in range(NCORES):
        n, rh = divmod(core, 2)
        r0 = rh * ROWS_PER_CORE
        nr = min(SLAB_ROWS, DHW - r0)                            # 26 or 25
        slab = np.zeros((CIN, NPLANES, SLAB_ROWS, DHW), np.float32)
        slab[:, :, :nr] = x[n, :, :, r0:r0 + nr]
        in_maps.append({
            "xs": np.ascontiguousarray(
                slab.reshape(CIN, XS_COLS).astype(NP_MM)),
            "wt": wfull[n],
            "bt": bt,
        })
    return in_maps


def _gather(results):
    y = np.empty((N, COUT, DOUT, DOUT, DOUT), np.float32)
    for core in range(NCORES):
        n, rh = divmod(core, 2)
        r0 = rh * ROWS_PER_CORE
        arr = results[core]["y"].reshape(NG, 2, COUT, ROWS_PER_CORE, DHW)
        y[n, :, :, r0:r0 + ROWS_PER_CORE, :] = (
            arr[:, :, :, :, :DOUT]
            .transpose(2, 0, 1, 3, 4)
            .reshape(COUT, DOUT, ROWS_PER_CORE, DOUT))
    return y


def kernel(x, s, style_weight, style_bias, weight, bias):
    global LAST_RESULTS
    x = np.asarray(x, np.float32)
    s = np.asarray(s, np.float32)
    style_weight = np.asarray(style_weight, np.float32)
    style_bias = np.asarray(style_bias, np.float32)
    weight = np.asarray(weight, np.float32)
    bias = np.asarray(bias, np.float32)

    if "nc" not in _CACHE:
        _CACHE["nc"] = _build_bass()
    in_maps = _prep_in_maps(x, s, style_weight, style_bias, weight, bias)
    res = None
    for attempt in range(3):
        try:
            res = run_bass_kernel_spmd(_CACHE["nc"], in_maps, list(range(NCORES)))
            break
        except Exception:
            if attempt == 2:
                raise
            time.sleep(30)  # transient device wedge; recovers on its own
    LAST_RESULTS = res
    return _gather(res.results)


# revision 6
# speedup vs baseline: 1.3963x; 1.3963x over previous
"""ConvMod3d on 8 trn2 cores - 1D Winograd F(2,3) along the w axis.

Sharding: 8 shards = 4 samples x 2 h-row halves (balanced; each core does
all 46 output planes x 23 output rows). Style modulation/demodulation AND
the Winograd input transform run on host; bias is added on host during
gather (zero device cost).

F(2,3) on w: per 2-wide output tile t (inputs s0..s3 = cols 2t..2t+3):
  x~0 = s0-s2  x~1 = s1+s2  x~2 = s2-s1  x~3 = s1-s3     (host, numpy)
  w~0 = g0     w~1 = (g0+g1+g2)/2  w~2 = (g0-g1+g2)/2  w~3 = g2
  y_even = y~0+y~1+y~2      y_odd = y~1-y~2-y~3          (device, DVE)
This cuts PE MACs per output 1.5x on the kw axis: 27 taps -> 4 phases x 9
(kd,kh) taps = 36 phase-taps over a 23-tile free dim (vs 27 taps over 46
cols): 36x23 = 828 vs 27x46 = 1242 tap-cols, at the cost of 4 psum banks
per accumulation (one per phase) + a 4-op DVE inverse transform.

Per output plane and phase m: 3 c128 matmuls (kd0,kd1 dominoes at kh=0,1,2
on window xw[d,m] = x~m plane d | plane d+1) + 3 c64 (kd2 at kh=0,1,2 on
xw[d+2,m] lower). 24 slots per group of 2 planes (ci PE col strips), free
dim 23 rows x 23 tiles = 529, chunked 12+11 rows (276/253 cols).

Outputs: even cols in y[g,:,0:529], odd in y[g,:,529:]; host interleaves.
"""

import time

import numpy as np
import ml_dtypes

import concourse.bacc as bacc
import concourse.bass as bass
import concourse.tile as tile
from concourse import mybir
from concourse.bass_utils import run_bass_kernel_spmd

EPS = 1e-8
N, CIN, COUT = 4, 64, 64
DHW, K = 48, 3
DOUT = DHW - K + 1          # 46
ROWS_PER_CORE = 23
SLAB_ROWS = ROWS_PER_CORE + K  # 26 input rows per plane slab
NPLANES = DHW               # 48
NT = 23                     # w tiles (2 out cols each)
NM = 4                      # winograd phases
XTP = SLAB_ROWS * NT        # 598 cols per (plane, phase)
XS_COLS = NPLANES * NM * XTP
FREE = ROWS_PER_CORE * NT   # 529 psum cols per plane
CHUNKS = [(0, 12), (12, 11)]  # (row0, nrows): 276 + 253 cols
NG = DOUT // 2              # 23 groups
NBLK = NM * 6               # 24 weight blocks (per m: 3 c128 + 3 c64)
NCORES = 8
LOOKAHEAD = 4

F32 = mybir.dt.float32
MM_DT = mybir.dt.bfloat16
NP_MM = np.dtype(ml_dtypes.bfloat16)

_CACHE = {}
LAST_RESULTS = None


def _xt_base(p, m):
    return (p * NM + m) * XTP


def _build_bass():
    nc = bacc.Bacc()
    xt = nc.declare_dram_parameter("xt", [CIN, XS_COLS], MM_DT, isOutput=False)
    wt = nc.declare_dram_parameter("wt", [128, NBLK * COUT], MM_DT, isOutput=False)
    # bf16 output halves the scalar-ring DMA bytes; host casts back
    y = nc.declare_dram_parameter("y", [NG, 128, 2 * FREE], MM_DT, isOutput=True)

    with tile.TileContext(nc) as tc:
        with (
            tc.tile_pool(name="const", bufs=1) as cpool,
            tc.tile_pool(name="xwpool", bufs=14) as xwpool,
            tc.tile_pool(name="tpool", bufs=16) as tpool,
            tc.tile_pool(name="opool", bufs=4) as opool,
            tc.tile_pool(name="ppool", bufs=8, space="PSUM") as ppool,
        ):
            wtile = cpool.tile([128, NBLK * COUT], MM_DT)
            nc.sync.dma_start(out=wtile[:, :], in_=wt[:, :])

            XW = {}

            def load_xw(d, split=False):
                # one tile holds ALL 4 phases of a plane pair (phases are
                # contiguous in xt), so each half loads as a single DMA
                # with 4784B partition lines - the per-phase [64,598]
                # windows (1196B lines) ran the rings at ~half throughput
                # and starved the PE. Lowers on sync ring, uppers+output
                # on scalar ring.
                if d in XW or d >= NPLANES:
                    return
                t = xwpool.tile([128, NM * XTP], MM_DT, tag="xw", name="xw")
                cuts = [0, 2 * XTP, NM * XTP] if split else [0, NM * XTP]
                base = _xt_base(d, 0)
                for a, b in zip(cuts, cuts[1:]):
                    nc.sync.dma_start(out=t[0:64, a:b],
                                      in_=xt[:, base + a:base + b])
                    if d + 1 < NPLANES:
                        nc.scalar.dma_start(
                            out=t[64:128, a:b],
                            in_=xt[:, base + NM * XTP + a:base + NM * XTP + b])
                XW[d] = t

            def ensure_group(g, split=False):
                if g >= NG:
                    return
                for d in (2 * g, 2 * g + 1, 2 * g + 2):
                    load_xw(d, split=split)

            ensure_group(0, split=True)
            for g0 in range(1, LOOKAHEAD):
                ensure_group(g0)

            for g in range(NG):
                ensure_group(g + LOOKAHEAD)
                ot = opool.tile([128, 2 * FREE], MM_DT, tag="ot")
                for cidx, (r0, nr) in enumerate(CHUNKS):
                    c0, ncols = r0 * NT, nr * NT
                    # Two passes per chunk with a partial inverse between:
                    # pass A accumulates phases m1,m0 then reduces them to
                    # SBUF (freeing those banks mid-chunk), pass B does
                    # m2,m3 and the final combine. Short accumulation
                    # blocks whose psum-free sems arrive late put ~380ns
                    # gaps at every block start, and ANY gap drops the PE
                    # to mid p-state (1.2GHz, 0.83ns/row) for the next 3us
                    # - which is how run A ran at exactly half rate.
                    ps = {m: ppool.tile([128, 512], F32, tag="ps", name="ps")
                          for m in (1, 0, 2, 3)}

                    def emit(morder, ppos):
                        border = []
                        for pos, m in enumerate(morder):
                            iorder = list(range(6))
                            if (ppos + pos) % 2 == 1:
                                iorder = iorder[::-1]
                            border += [(m, i) for i in iorder]
                        mms = [(m, i, ci) for (m, i) in border
                               for ci in range(2)]
                        first, last = {}, {}
                        for idx, (m, i, ci) in enumerate(mms):
                            first.setdefault((m, ci), idx)
                            last[(m, ci)] = idx
                        for idx, (m, i, ci) in enumerate(mms):
                            d = 2 * g + ci
                            j = m * 6 + i
                            if i < 3:
                                win, off = XW[d], m * XTP + i * NT + c0
                                lo, hi = 0, 128
                            else:
                                # c64 monos as DIAGONAL PE quadrants:
                                # ci0 tile (0,0) reads XW[2g+2] lower
                                # (plane 2g+2); ci1 tile (64,64) reads
                                # XW[2g+2] UPPER (plane 2g+3, already
                                # loaded; weight block duplicated at
                                # partitions 64:128).
                                win = XW[2 * g + 2]
                                off = m * XTP + (i - 3) * NT + c0
                                lo, hi = ci * 64, (ci + 1) * 64
                            nc.tensor.matmul(
                                ps[m][ci * 64:(ci + 1) * 64, 0:ncols],
                                wtile[lo:hi, j * 64:(j + 1) * 64],
                                win[lo:hi, off:off + ncols],
                                start=(idx == first[(m, ci)]),
                                stop=(idx == last[(m, ci)]),
                            )

                    emit((1, 0), 0)
                    # partial inverse (overlaps pass B's matmuls):
                    # t1 = m1 (scalar stages psum->SBUF), e01 = m0+m1
                    t1 = tpool.tile([128, 276], F32, tag="tmp", name="tmp")
                    nc.scalar.copy(t1[:, 0:ncols], ps[1][:, 0:ncols])
                    e01 = tpool.tile([128, 276], F32, tag="tmp", name="tmp")
                    nc.vector.tensor_add(out=e01[:, 0:ncols],
                                         in0=t1[:, 0:ncols],
                                         in1=ps[0][:, 0:ncols])
                    emit((2, 3), 2)
                    # final combine: even = e01+m2, odd = (m1-m2)-m3
                    nc.vector.tensor_add(out=ot[:, c0:c0 + ncols],
                                         in0=e01[:, 0:ncols],
                                         in1=ps[2][:, 0:ncols])
                    t3 = tpool.tile([128, 276], F32, tag="tmp", name="tmp")
                    nc.vector.tensor_sub(out=t3[:, 0:ncols],
                                         in0=t1[:, 0:ncols],
                                         in1=ps[2][:, 0:ncols])
                    nc.vector.tensor_sub(out=ot[:, FREE + c0:FREE + c0 + ncols],
                                         in0=t3[:, 0:ncols],
                                         in1=ps[3][:, 0:ncols])
                nc.scalar.dma_start(out=y[g, :, :], in_=ot[:, :])
    nc.compile()
    return nc


def _prep_in_maps(x, s, style_weight, style_bias, weight, bias):
    style = s @ style_weight.T + style_bias
    wm = weight[None] * style[:, None, :, None, None, None]
    wm = wm * (1.0 / np.sqrt((wm * wm).sum(axis=(2, 3, 4, 5), keepdims=True) + EPS))
    wk = wm.transpose(0, 2, 3, 4, 5, 1)            # [N,Ci,kd,kh,kw,Co]
    a, b, c = wk[:, :, :, :, 0, :], wk[:, :, :, :, 1, :], wk[:, :, :, :, 2, :]
    wtil = [a, (a + b + c) * 0.5, (a - b + c) * 0.5, c]   # per-phase [N,Ci,kd,kh,Co]
    wfull = np.zeros((N, 128, NBLK * COUT), np.float32)
    for m in range(NM):
        for i in range(3):
            j = m * 6 + i
            wfull[:, 0:64, j * 64:(j + 1) * 64] = wtil[m][:, :, 0, i, :]
            wfull[:, 64:128, j * 64:(j + 1) * 64] = wtil[m][:, :, 1, i, :]
        for i in range(3):
            j = m * 6 + 3 + i
            wfull[:, 0:64, j * 64:(j + 1) * 64] = wtil[m][:, :, 2, i, :]
            wfull[:, 64:128, j * 64:(j + 1) * 64] = wtil[m][:, :, 2, i, :]
    wfull = np.ascontiguousarray(wfull.astype(NP_MM))

    in_maps = []
    for core in range(NCORES):
        n, rh = divmod(core, 2)
        r0 = rh * ROWS_PER_CORE
        nr = min(SLAB_ROWS, DHW - r0)
        slab = np.zeros((CIN, NPLANES, SLAB_ROWS, DHW), np.float32)
        slab[:, :, :nr] = x[n, :, :, r0:r0 + nr]
        s0 = slab[..., 0:46:2]
        s1 = slab[..., 1:47:2]
        s2 = slab[..., 2:48:2]
        s3 = slab[..., 3:48:2]
        xtil = np.stack([s0 - s2, s1 + s2, s2 - s1, s1 - s3], axis=2)
        # [Ci, P, 4, R, 23] -> [Ci, P*4*598]
        in_maps.append({
            "xt": np.ascontiguousarray(
                xtil.reshape(CIN, XS_COLS).astype(NP_MM)),
            "wt": wfull[n],
        })
    return in_maps


def _gather(results, bias):
    y = np.empty((N, COUT, DOUT, DOUT, DOUT), np.float32)
    for core in range(NCORES):
        n, rh = divmod(core, 2)
        r0 = rh * ROWS_PER_CORE
        res = results[core]["y"]                       # [NG, 128, 1058]
        full = np.empty((NG, 2, COUT, ROWS_PER_CORE, DOUT), np.float32)
        full[..., 0::2] = res[:, :, 0:FREE].reshape(
            NG, 2, COUT, ROWS_PER_CORE, NT)
        full[..., 1::2] = res[:, :, FREE:].reshape(
            NG, 2, COUT, ROWS_PER_CORE, NT)
        y[n, :, :, r0:r0 + ROWS_PER_CORE, :] = (
            full.transpose(2, 0, 1, 3, 4).reshape(
                COUT, DOUT, ROWS_PER_CORE, DOUT))
    return y + bias[None, :, None, None, None]


def kernel(x, s, style_weight, style_bias, weight, bias):
    global LAST_RESULTS
    x = np.asarray(x, np.float32)
    s = np.asarray(s, np.float32)
    style_weight = np.asarray(style_weight, np.float32)
    style_bias = np.asarray(style_bias, np.float32)
    weight = np.asarray(weight, np.float32)
    bias = np.asarray(bias, np.float32)

    if "nc" not in _CACHE:
        _CACHE["nc"] = _build_bass()
    in_maps = _prep_in_maps(x, s, style_weight, style_bias, weight, bias)
    res = None
    for attempt in range(3):
        try:
            res = run_bass_kernel_spmd(_CACHE["nc"], in_maps, list(range(NCORES)))
            break
        except Exception:
            if attempt == 2:
                raise
            time.sleep(30)
    LAST_RESULTS = res
    return _gather(res.results, bias)
